# revision 22
# baseline (speedup 1.0000x reference)
"""Trainium2 Bass kernel for nn_CapsuleLayer_45148696216021.

Mathematical structure (verified against the reference):
  caps = einsum('bi,nio->bno', x, rel_W) + rel_b          [B, N, O]
  caps_t[b] = caps[b].T.reshape(N, O)  (torch view quirk)
  u_hat[b,i,n] = sum_o caps_t[b,n,o] * rw[b,i,o]
  Dynamic routing with b_logits starting at 0: softmax over the capsule
  axis of a tensor whose rows (capsule axis) are identical stays exactly
  uniform (1/N) at EVERY iteration, because the agreement update
  b += einsum('bik,bjk->bji', u_hat, v) is j-independent when v rows are
  identical.  Hence the output v[b,j,:] == squash(sum_i u_hat[b,i,:]/N)
  for all j (bitwise identical rows in the reference too).

  sum_i u_hat[b,i,n] = sum_o caps_t[b,n,o] * rwsum[b,o]
  with rwsum[b,o] = sum_i rw[b,i,o].  Substituting the caps_t view:
  su[b,n] = sum_{r,m} caps[b,r,8n+m] * rwsum[b, m*128+r]

  So the only heavy compute is caps = x @ rel_W (34 GFLOP over 512 MB of
  weights), followed by a cheap weighted reduction.  rwsum and the rel_b
  bias contribution are tiny and computed on the host.

Sharding: the O axis (1024) is split into 8 slices of 128 columns; core d
computes caps[:, :, 128d:128d+128] for all relations, then reduces with
the rwsum weights to su[:, 16d:16d+16] fully on-chip (capsule n uses
exactly caps columns 8n..8n+7, which lie entirely in one slice).  The
only device output is su (8 KB/core); host applies bias + squash +
row-broadcast to the [128,128,128] output.

Precision: weights stream as TRN float8e3 (E3M4, 4 mantissa bits),
pre-scaled by W_SCALE so sigma ~ 2-4 sits in the normal range (max
15.5); the 1/W_SCALE is folded into the rwsv multiplier.  x stays bf16
(the PE upconverts both operands to ~fp22 internally, mixed dtypes are
allowed).  This halves the HBM weight traffic, which is the bottleneck.
"""

import os
import sys
import tempfile
from concurrent.futures import ThreadPoolExecutor

import numpy as np

if "/opt/trn_rl_repo" not in sys.path:
    sys.path.insert(0, "/opt/trn_rl_repo")

import concourse.bass as bass
import concourse.mybir as mybir
import concourse.tile as tile
from concourse.vector_clock import ScopedClock
from concourse import bass_utils
from concourse.bass_utils import run_bass_kernel_spmd

if os.environ.get("BASS_LDW_OPT", "0") == "1":
    _orig_run_command = bass_utils.run_command

    def _patched_run_command(argv, **kw):
        argv = [
            "--enable-ldw-opt=true" if a == "--enable-ldw-opt=false" else a
            for a in argv
        ]
        return _orig_run_command(argv, **kw)

    bass_utils.run_command = _patched_run_command

B, I, O, N = 128, 1024, 1024, 128
NC = 8          # cores
G = 32          # relation groups of 4
CSL = O // NC   # 128 c-columns per core

import ml_dtypes

_W_DT_NAME = os.environ.get("BASS_W_DTYPE", "float8e3")
_W_DT = getattr(mybir.dt, _W_DT_NAME)
_W_SCALE = float(os.environ.get("BASS_W_SCALE", "64"))
_RW_BF16 = os.environ.get("BASS_RW_BF16", "1") == "1"

LAST_RESULTS = None  # stashed BassKernelResults for test.py introspection


def _ensure_ntff_hook():
    """This image's antenv package lacks axon_hooks (bass_utils crashes on
    the import when trace=True).  Provide the module and register the same
    ctypes-based NTFF hook trn_boot would have installed."""
    try:
        from antenv.axon_hooks import get_axon_ntff_profile_hook  # noqa: F401

        return
    except ImportError:
        pass
    import types

    import antenv
    from trn_agent_boot.trn_boot import _ntff_profile_via_ctypes

    mod = types.ModuleType("antenv.axon_hooks")
    _h = [None]
    mod.set_axon_ntff_profile_hook = lambda h: _h.__setitem__(0, h)
    mod.get_axon_ntff_profile_hook = lambda: _h[0]
    sys.modules["antenv.axon_hooks"] = mod
    antenv.axon_hooks = mod
    mod.set_axon_ntff_profile_hook(
        _ntff_profile_via_ctypes("/opt/axon/libaxon_pjrt.so")
    )


def _cheap_tail(self, tick_clock, wait_clock):
    """Minimal Tile kernel tail: gpsimd observes the global clock via a NOP
    wait chain (split to single waits later), then resets the semaphores for
    re-execution.  No drains / all-engine barriers: every proc's final tick
    is in the global clock, so nothing can touch a semaphore afterwards."""
    carrier = self.nc.gpsimd.nop(nofuse=True)
    wait_clock.add_sem_waits(
        carrier.ins, ScopedClock({None: tick_clock.global_clock})
    )
    popped = self.nc._tile_sem_poison_stack.pop()
    assert popped is self._sem_poison
    self.nc.clear_and_free_semaphores(list(self.sems.allocated().values()))


tile.TileContext._drain_and_barrier = _cheap_tail


def _strip_framework_overhead(nc):
    """Remove the bass preamble all-engine barrier + per-engine drains (a
    single-shot kernel reading no const-APs doesn't need them).  The
    reset-sema drain / range-clear of the tail is kept for re-execution."""
    n = 0
    for f in nc.m.functions:
        for blk in f.blocks:
            keep = []
            for inst in blk.instructions:
                tn = type(inst).__name__
                drop = False
                if tn == "InstDrain" and inst.reset_range_start is None:
                    drop = True
                elif tn == "InstEventSemaphore" and inst.name.startswith(
                    "barrier_"
                ):
                    drop = True
                if drop:
                    n += 1
                else:
                    keep.append(inst)
            blk.instructions = keep
    return n


def _split_multi_waits(nc):
    """This walrus build only supports one semaphore wait per instruction.
    Tile's wait-assigner can attach several; split the extras onto
    same-engine NOPs inserted immediately before the instruction (same
    semantics: the engine blocks on each wait in turn)."""
    n_split = 0
    for f in nc.m.functions:
        for blk in f.blocks:
            new = []
            dirty = False
            for inst in blk.instructions:
                si = inst.sync_info
                waits = list(si.on_wait) if si is not None else []
                if len(waits) > 1:
                    dirty = True
                    n_split += 1
                    for w in waits[:-1]:
                        nop = mybir.InstNoOp(
                            name=nc.get_next_instruction_name(), ins=[], outs=[]
                        )
                        nop.engine = inst.engine
                        nop.sync_info = mybir.SyncInfo(on_wait=[w], on_update=[])
                        new.append(nop)
                    inst.sync_info = mybir.SyncInfo(
                        on_wait=[waits[-1]], on_update=list(si.on_update)
                    )
                new.append(inst)
            if dirty:
                blk.instructions = new
    return n_split

_NC_CACHE = {}
_F_PRE = int(os.environ.get("BASS_F_PRE", "26"))
_F_MID = int(os.environ.get("BASS_F_MID", "0"))


class _WeightedQueue:
    """Byte-weighted round-robin across the 3 DMA queues, proportional to
    measured queue rates (sync/scalar HWDGE ~111 GB/s, gpsimd SWDGE ~94)."""

    RATES = (111.0, 111.0, 94.0)

    def __init__(self, preload_bytes=(0.0, 0.0, 0.0)):
        # busy-time accumulated per queue (bytes / rate)
        self.t = [pb / r for pb, r in zip(preload_bytes, self.RATES)]

    def pick(self, nbytes):
        q = min(range(3), key=lambda i: self.t[i] + nbytes / self.RATES[i])
        self.t[q] += nbytes / self.RATES[q]
        return q


def _build_bass():
    """Per-core program: caps matmul over this core's c-slice + weighted
    reduction to su[:, 16 local capsules]."""
    key = (_W_DT_NAME, _F_PRE, _F_MID, _RW_BF16)
    if key in _NC_CACHE:
        return _NC_CACHE[key]

    f32 = mybir.dt.float32
    bf16 = mybir.dt.bfloat16
    rw_dt = bf16 if _RW_BF16 else f32
    nc = bass.Bass("TRN2", target_bir_lowering=False)
    xt_d = nc.declare_dram_parameter("xt", [128, 8, 128], bf16, isOutput=False)
    w_d = nc.declare_dram_parameter("w", [G, 128, 4, 8, CSL], _W_DT, isOutput=False)
    # rwsv in [b, g, r4, m] layout: contiguous (r4, m) per group
    rw_d = nc.declare_dram_parameter("rwsv", [128, G, 4, 8], rw_dt, isOutput=False)
    # two half-sums (host adds them): lets the first reduce+DMA overlap the
    # final group's matmuls instead of serializing after them
    su_d = nc.declare_dram_parameter("su", [2, 128, 16], f32, isOutput=True)

    with tile.TileContext(nc) as tc:
        with (
            tc.tile_pool(name="const", bufs=1) as cpool,
            tc.tile_pool(name="wts", bufs=12) as wpool,
            tc.tile_pool(name="tmp", bufs=3) as tpool,
            tc.tile_pool(name="ps", bufs=6, space="PSUM") as pspool,
            tc.tile_pool(name="warmp", bufs=1, space="PSUM") as warmpool,
        ):
            # Warm-up fillers are fed from a memset tile so they have no DMA
            # dependency and can start at t~0 (keeps HAM warming while the
            # lead-in DMAs stream).
            warm_in = cpool.tile([128, 256], bf16)
            nc.vector.memset(warm_in[:], 0.0)
            warm = warmpool.tile([128, 256], f32, tag="warm")

            def fillers(n):
                for _ in range(n):
                    nc.tensor.matmul(warm[:], warm_in[:, 0:128], warm_in[:])

            fillers(_F_PRE)

            # Lead-in: groups 0-4 are half-gated (two 2-relation tiles, each
            # matmul-able as soon as its own 256 KB DMA lands), with an
            # explicit queue schedule that minimizes the PE's early stalls.
            # rwsv trails on scalar (the vector engine has plenty of slack).
            LEADIN = 5
            xt = cpool.tile([128, 8, 128], bf16)
            half = {}  # (g, 0/1) -> [128, 2, 8, CSL] tile
            for g in range(LEADIN):
                for h in range(2):
                    half[(g, h)] = wpool.tile(
                        [128, 2, 8, CSL], _W_DT, tag="wt", name=f"wl{g}_{h}"
                    )

            def dma_half(eng, g, h):
                eng.dma_start(half[(g, h)][:], w_d[g, :, 2 * h : 2 * h + 2])

            rw = cpool.tile([128, G, 4, 8], rw_dt)
            # sync:   xt, w0b, w3a, w4a
            nc.sync.dma_start(xt[:], xt_d[:])
            dma_half(nc.sync, 0, 1)
            dma_half(nc.sync, 3, 0)
            dma_half(nc.sync, 4, 0)
            # scalar: w0a, w1a, w2a, w3b, rw
            dma_half(nc.scalar, 0, 0)
            dma_half(nc.scalar, 1, 0)
            dma_half(nc.scalar, 2, 0)
            dma_half(nc.scalar, 3, 1)
            nc.scalar.dma_start(rw[:], rw_d[:])
            # gpsimd: w1b, w2b, w4b
            dma_half(nc.gpsimd, 1, 1)
            dma_half(nc.gpsimd, 2, 1)
            dma_half(nc.gpsimd, 4, 1)

            # paired accumulator: [128, 2, 4, 16, 8] so a 2-group product
            # pair is absorbed in ONE 1024-element add.  All adds stay on the
            # vector engine: gpsimd is strict-FIFO, so an add queued there
            # blocks the weight-DMA issues behind it and stalls the PE.
            acc_v = cpool.tile([128, 2, 4, 16, 8], f32)
            nc.vector.memset(acc_v[:], 0.0)

            # lead-in bytes already committed per queue (for the balancer)
            half_b = 128 * 2 * 8 * CSL * (1 if _W_DT_NAME == "float8e3" else 2)
            xt_b = 128 * 8 * 128 * 2
            rw_b = 128 * G * 4 * 8 * (2 if _RW_BF16 else 4)
            wq = _WeightedQueue(
                (xt_b + 3 * half_b, 4 * half_b + rw_b, 3 * half_b)
            )
            dma_engines = [nc.sync, nc.scalar, nc.gpsimd]

            def mult(dst, ps_ap, g, nrel=4, rel0=0):
                # dst = ps * rwsv[b, g, rel0:rel0+nrel, m] (broadcast over nl)
                in1 = rw[:, g, rel0 : rel0 + nrel]
                in1 = in1[:, :, None, :].to_broadcast([128, nrel, 16, 8])
                nc.vector.tensor_tensor(dst, ps_ap, in1, mybir.AluOpType.mult)

            # pair p = (g=2p, g=2p+1); the last pair is two solo adds so the
            # tail chain after the final matmul stays short.
            pair_t = None

            for g in range(G):
                slot = g % 2
                if slot == 0:
                    pair_t = tpool.tile([128, 2, 4, 16, 8], f32, tag="tmp")
                if g < LEADIN:
                    for h in range(2):
                        ps2 = pspool.tile(
                            [128, 2, 16, 8], f32, tag="ps", name=f"psl{g}_{h}"
                        )
                        for k in range(8):
                            nc.tensor.matmul(
                                ps2[:],
                                xt[:, k, :],
                                half[(g, h)][:, :, k, :],
                                start=(k == 0),
                                stop=(k == 7),
                            )
                        mult(pair_t[:, slot, 2 * h : 2 * h + 2], ps2[:], g, 2, 2 * h)
                else:
                    # one 1 MB transfer per TWO groups: quarter the DMA
                    # issues / completion semaphores of the two-half scheme
                    # (the deep prefetch makes coarse gating irrelevant
                    # mid-kernel; per-partition runs stay 4 KB contiguous)
                    off = (g - LEADIN) % 2
                    if off == 0:
                        npair = 2 if g + 1 < G else 1
                        wt2 = wpool.tile(
                            [128, npair, 4, 8, CSL], _W_DT, tag="wt"
                        )
                        q = dma_engines[wq.pick(npair * 2 * half_b)]
                        q.dma_start(
                            wt2[:],
                            w_d[g : g + npair].transpose([1, 0, 2, 3, 4]),
                        )
                    ps = pspool.tile([128, 4, 16, 8], f32, tag="ps")
                    for k in range(8):
                        nc.tensor.matmul(
                            ps[:],
                            xt[:, k, :],
                            wt2[:, off, :, k, :],
                            start=(k == 0),
                            stop=(k == 7),
                        )
                    mult(pair_t[:, slot], ps[:], g)
                p = g // 2
                if p == G // 2 - 1:
                    # last pair: solo adds into separate slots; slot 0's
                    # reduce + DMA overlap the final group's matmuls
                    nc.vector.tensor_tensor(
                        acc_v[:, slot], acc_v[:, slot], pair_t[:, slot],
                        mybir.AluOpType.add,
                    )
                    su_t = cpool.tile([128, 16], f32, name=f"su{slot}")
                    nc.vector.tensor_reduce(
                        su_t[:],
                        acc_v[:, slot].transpose([0, 2, 1, 3]),
                        mybir.AxisListType.XY,
                        mybir.AluOpType.add,
                    )
                    (nc.sync if slot == 0 else nc.scalar).dma_start(
                        su_d[slot], su_t[:]
                    )
                elif slot == 1:
                    nc.vector.tensor_tensor(
                        acc_v[:], acc_v[:], pair_t[:], mybir.AluOpType.add
                    )
                if g < G - 1:
                    fillers(_F_MID)

    if os.environ.get("BASS_STRIP_FRAMEWORK", "1") == "1":
        _strip_framework_overhead(nc)
    _split_multi_waits(nc)
    _NC_CACHE[key] = nc
    return nc


def _to_bf16(a):
    """Fast float32 -> bfloat16 with round-to-nearest-even (numpy bit ops;
    ml_dtypes astype is ~50x slower)."""
    u = a.view(np.uint32)
    r = ((u >> 16) & 1) + np.uint32(0x7FFF)
    return ((u + r) >> 16).astype(np.uint16).view(ml_dtypes.bfloat16)


def _quant_w(rel_W):
    """rel_W [N, I, O] f32 -> scaled float8e3 (or bf16 fallback)."""
    if _W_DT_NAME == "float8e3":
        z = np.clip(rel_W * np.float32(_W_SCALE), -15.5, 15.5)
        return z.astype(ml_dtypes.float8_e3m4)
    return _to_bf16(np.ascontiguousarray(rel_W, np.float32))


def _prep_core_w(w6, d):
    # w6: [G, 4, 8, 128, NC, CSL] quantized view -> (g, i_loc, r4, k, c)
    return np.ascontiguousarray(w6[:, :, :, :, d, :].transpose(0, 3, 1, 2, 4))


def kernel(x, edge_index, edge_type, rel_W, rel_b, route_weights):
    global LAST_RESULTS
    x = np.asarray(x, np.float32)
    rel_W = np.asarray(rel_W, np.float32)
    rel_b = np.asarray(rel_b, np.float32)
    rw = np.asarray(route_weights, np.float32).reshape(B, I, O)

    # host-side tiny reductions
    rwsum = rw.sum(axis=1, dtype=np.float32)                # [B, O]
    rwsv = np.ascontiguousarray(rwsum.reshape(B, 8, 128))   # [b, m, r]
    bias2 = np.einsum(
        "rnm,bmr->bn", rel_b.reshape(N, N, 8), rwsv, optimize=True
    )  # [B, N]

    # device input prep
    xt = _to_bf16(np.ascontiguousarray(x.reshape(B, 8, 128).transpose(2, 1, 0)))
    wq8 = _quant_w(rel_W)
    w6 = wq8.reshape(G, 4, 8, 128, NC, CSL)  # (g, r4, k, i_loc, d, c)
    with ThreadPoolExecutor(NC) as ex:
        w_cores = list(ex.map(lambda d: _prep_core_w(w6, d), range(NC)))

    # device rwsv in [b, g, r4, m] layout, with the weight scale folded in
    rwt = rwsum.reshape(B, 8, G, 4).transpose(0, 2, 3, 1)  # [b, g, r4, m]
    rwsv_dev = rwt * np.float32(1.0 / _W_SCALE if _W_DT_NAME == "float8e3" else 1.0)
    if _RW_BF16:
        rwsv_dev = _to_bf16(np.ascontiguousarray(rwsv_dev))
    else:
        rwsv_dev = np.ascontiguousarray(rwsv_dev)

    nc = _build_bass()
    in_maps = [{"xt": xt, "w": w_cores[d], "rwsv": rwsv_dev} for d in range(NC)]
    trace = bool(int(os.environ.get("KERNEL_TRACE", "0")))
    kwargs = {}
    if trace:
        _ensure_ntff_hook()
        kwargs["tmpdir"] = os.environ.get("KERNEL_TRACE_DIR") or tempfile.mkdtemp(
            prefix="capsule_trace_"
        )
    res = run_bass_kernel_spmd(nc, in_maps, list(range(NC)), trace=trace, **kwargs)
    LAST_RESULTS = res

    su = np.concatenate(
        [res.results[d]["su"].sum(axis=0, dtype=np.float32) for d in range(NC)],
        axis=1,
    )  # [B, N]
    su += bias2

    s = su * np.float32(1.0 / N)
    sn = np.sum(s * s, axis=-1, keepdims=True)
    vrow = (sn / (1.0 + sn) * s / np.sqrt(sn)).astype(np.float32)  # [B, N]
    out = np.empty((B, N, N), np.float32)
    out[:] = vrow[:, None, :]
    return out


# revision 24
# speedup vs baseline: 1.1676x; 1.1676x over previous
"""Trainium2 Bass kernel for nn_CapsuleLayer_45148696216021.

Mathematical structure (verified against the reference):
  caps = einsum('bi,nio->bno', x, rel_W) + rel_b          [B, N, O]
  caps_t[b] = caps[b].T.reshape(N, O)  (torch view quirk)
  u_hat[b,i,n] = sum_o caps_t[b,n,o] * rw[b,i,o]
  Dynamic routing with b_logits starting at 0: softmax over the capsule
  axis of a tensor whose rows (capsule axis) are identical stays exactly
  uniform (1/N) at EVERY iteration, because the agreement update
  b += einsum('bik,bjk->bji', u_hat, v) is j-independent when v rows are
  identical.  Hence the output v[b,j,:] == squash(sum_i u_hat[b,i,:]/N)
  for all j (bitwise identical rows in the reference too).

  sum_i u_hat[b,i,n] = sum_o caps_t[b,n,o] * rwsum[b,o]
  with rwsum[b,o] = sum_i rw[b,i,o].  Substituting the caps_t view:
  su[b,n] = sum_{r,m} caps[b,r,8n+m] * rwsum[b, m*128+r]

  So the only heavy compute is caps = x @ rel_W (34 GFLOP over 512 MB of
  weights), followed by a cheap weighted reduction.  rwsum and the rel_b
  bias contribution are tiny and computed on the host.

Sharding: the O axis (1024) is split into 8 slices of 128 columns; core d
computes caps[:, :, 128d:128d+128] for all relations, then reduces with
the rwsum weights to su[:, 16d:16d+16] fully on-chip (capsule n uses
exactly caps columns 8n..8n+7, which lie entirely in one slice).  The
only device output is su (8 KB/core); host applies bias + squash +
row-broadcast to the [128,128,128] output.

Precision: weights stream as TRN float8e3 (E3M4, 4 mantissa bits),
pre-scaled by W_SCALE so sigma ~ 2-4 sits in the normal range (max
15.5); the 1/W_SCALE is folded into the rwsv multiplier.  x stays bf16
(the PE upconverts both operands to ~fp22 internally, mixed dtypes are
allowed).  This halves the HBM weight traffic, which is the bottleneck.
"""

import os
import sys
import tempfile
from concurrent.futures import ThreadPoolExecutor

import numpy as np

if "/opt/trn_rl_repo" not in sys.path:
    sys.path.insert(0, "/opt/trn_rl_repo")

import concourse.bass as bass
import concourse.mybir as mybir
import concourse.tile as tile
from concourse.vector_clock import ScopedClock
from concourse import bass_utils
from concourse.bass_utils import run_bass_kernel_spmd

if os.environ.get("BASS_LDW_OPT", "0") == "1":
    _orig_run_command = bass_utils.run_command

    def _patched_run_command(argv, **kw):
        argv = [
            "--enable-ldw-opt=true" if a == "--enable-ldw-opt=false" else a
            for a in argv
        ]
        return _orig_run_command(argv, **kw)

    bass_utils.run_command = _patched_run_command

B, I, O, N = 128, 1024, 1024, 128
NC = 8          # cores
G = 32          # relation groups of 4
CSL = O // NC   # 128 c-columns per core

import ml_dtypes

_W_DT_NAME = os.environ.get("BASS_W_DTYPE", "float8e3")
_W_DT = getattr(mybir.dt, _W_DT_NAME)
_W_SCALE = float(os.environ.get("BASS_W_SCALE", "64"))
_RW_BF16 = os.environ.get("BASS_RW_BF16", "1") == "1"

LAST_RESULTS = None  # stashed BassKernelResults for test.py introspection


def _ensure_ntff_hook():
    """This image's antenv package lacks axon_hooks (bass_utils crashes on
    the import when trace=True).  Provide the module and register the same
    ctypes-based NTFF hook trn_boot would have installed."""
    try:
        from antenv.axon_hooks import get_axon_ntff_profile_hook  # noqa: F401

        return
    except ImportError:
        pass
    import types

    import antenv
    from trn_agent_boot.trn_boot import _ntff_profile_via_ctypes

    mod = types.ModuleType("antenv.axon_hooks")
    _h = [None]
    mod.set_axon_ntff_profile_hook = lambda h: _h.__setitem__(0, h)
    mod.get_axon_ntff_profile_hook = lambda: _h[0]
    sys.modules["antenv.axon_hooks"] = mod
    antenv.axon_hooks = mod
    mod.set_axon_ntff_profile_hook(
        _ntff_profile_via_ctypes("/opt/axon/libaxon_pjrt.so")
    )


def _cheap_tail(self, tick_clock, wait_clock):
    """Minimal Tile kernel tail: gpsimd observes the global clock via a NOP
    wait chain (split to single waits later), then resets the semaphores for
    re-execution.  No drains / all-engine barriers: every proc's final tick
    is in the global clock, so nothing can touch a semaphore afterwards."""
    carrier = self.nc.gpsimd.nop(nofuse=True)
    wait_clock.add_sem_waits(
        carrier.ins, ScopedClock({None: tick_clock.global_clock})
    )
    popped = self.nc._tile_sem_poison_stack.pop()
    assert popped is self._sem_poison
    self.nc.clear_and_free_semaphores(list(self.sems.allocated().values()))


tile.TileContext._drain_and_barrier = _cheap_tail


def _strip_framework_overhead(nc):
    """Remove the bass preamble all-engine barrier + per-engine drains (a
    single-shot kernel reading no const-APs doesn't need them).  The
    reset-sema drain / range-clear of the tail is kept for re-execution."""
    n = 0
    for f in nc.m.functions:
        for blk in f.blocks:
            keep = []
            for inst in blk.instructions:
                tn = type(inst).__name__
                drop = False
                if tn == "InstDrain" and inst.reset_range_start is None:
                    drop = True
                elif tn == "InstEventSemaphore" and inst.name.startswith(
                    "barrier_"
                ):
                    drop = True
                if drop:
                    n += 1
                else:
                    keep.append(inst)
            blk.instructions = keep
    return n


def _split_multi_waits(nc):
    """This walrus build only supports one semaphore wait per instruction.
    Tile's wait-assigner can attach several; split the extras onto
    same-engine NOPs inserted immediately before the instruction (same
    semantics: the engine blocks on each wait in turn)."""
    n_split = 0
    for f in nc.m.functions:
        for blk in f.blocks:
            new = []
            dirty = False
            for inst in blk.instructions:
                si = inst.sync_info
                waits = list(si.on_wait) if si is not None else []
                if len(waits) > 1:
                    dirty = True
                    n_split += 1
                    for w in waits[:-1]:
                        nop = mybir.InstNoOp(
                            name=nc.get_next_instruction_name(), ins=[], outs=[]
                        )
                        nop.engine = inst.engine
                        nop.sync_info = mybir.SyncInfo(on_wait=[w], on_update=[])
                        new.append(nop)
                    inst.sync_info = mybir.SyncInfo(
                        on_wait=[waits[-1]], on_update=list(si.on_update)
                    )
                new.append(inst)
            if dirty:
                blk.instructions = new
    return n_split

_NC_CACHE = {}
_F_PRE = int(os.environ.get("BASS_F_PRE", "26"))
_F_MID = int(os.environ.get("BASS_F_MID", "0"))


class _WeightedQueue:
    """Byte-weighted round-robin across the 3 DMA queues, proportional to
    measured queue rates (sync/scalar HWDGE ~111 GB/s, gpsimd SWDGE ~94)."""

    RATES = (111.0, 111.0, 94.0)

    def __init__(self, preload_bytes=(0.0, 0.0, 0.0)):
        # busy-time accumulated per queue (bytes / rate)
        self.t = [pb / r for pb, r in zip(preload_bytes, self.RATES)]

    def pick(self, nbytes):
        q = min(range(3), key=lambda i: self.t[i] + nbytes / self.RATES[i])
        self.t[q] += nbytes / self.RATES[q]
        return q


def _build_bass():
    """Per-core program: caps matmul over this core's c-slice + weighted
    reduction to su[:, 16 local capsules]."""
    key = (_W_DT_NAME, _F_PRE, _F_MID, _RW_BF16)
    if key in _NC_CACHE:
        return _NC_CACHE[key]

    f32 = mybir.dt.float32
    bf16 = mybir.dt.bfloat16
    rw_dt = bf16 if _RW_BF16 else f32
    nc = bass.Bass("TRN2", target_bir_lowering=False)
    xt_d = nc.declare_dram_parameter("xt", [128, 8, 128], bf16, isOutput=False)
    w_d = nc.declare_dram_parameter("w", [G, 128, 4, 8, CSL], _W_DT, isOutput=False)
    # rwsv in [b, g, r4, m] layout: contiguous (r4, m) per group
    rw_d = nc.declare_dram_parameter("rwsv", [128, G, 4, 8], rw_dt, isOutput=False)
    # two half-sums (host adds them): lets the first reduce+DMA overlap the
    # final group's matmuls instead of serializing after them
    su_d = nc.declare_dram_parameter("su", [2, 128, 16], f32, isOutput=True)

    with tile.TileContext(nc) as tc:
        with (
            tc.tile_pool(name="const", bufs=1) as cpool,
            tc.tile_pool(name="wts", bufs=16) as wpool,
            tc.tile_pool(name="tmp", bufs=3) as tpool,
            tc.tile_pool(name="ps", bufs=6, space="PSUM") as pspool,
            tc.tile_pool(name="warmp", bufs=1, space="PSUM") as warmpool,
        ):
            # Warm-up fillers are fed from a memset tile so they have no DMA
            # dependency and can start at t~0 (keeps HAM warming while the
            # lead-in DMAs stream).
            warm_in = cpool.tile([128, 256], bf16)
            nc.vector.memset(warm_in[:], 0.0)
            warm = warmpool.tile([128, 256], f32, tag="warm")

            def fillers(n):
                for _ in range(n):
                    nc.tensor.matmul(warm[:], warm_in[:, 0:128], warm_in[:])

            fillers(_F_PRE)

            # Lead-in: groups 0-4 are half-gated (two 2-relation tiles, each
            # matmul-able as soon as its own 256 KB DMA lands), with an
            # explicit queue schedule that minimizes the PE's early stalls.
            # rwsv trails on scalar (the vector engine has plenty of slack).
            LEADIN = 5
            xt = cpool.tile([128, 8, 128], bf16)
            half = {}  # (g, 0/1) -> [128, 2, 8, CSL] tile
            for g in range(LEADIN):
                for h in range(2):
                    half[(g, h)] = wpool.tile(
                        [128, 2, 8, CSL], _W_DT, tag="wt", name=f"wl{g}_{h}"
                    )

            def dma_half(eng, g, h):
                eng.dma_start(half[(g, h)][:], w_d[g, :, 2 * h : 2 * h + 2])

            rw = cpool.tile([128, G, 4, 8], rw_dt)
            # sync:   xt, w0b, w3a, w4a
            nc.sync.dma_start(xt[:], xt_d[:])
            dma_half(nc.sync, 0, 1)
            dma_half(nc.sync, 3, 0)
            dma_half(nc.sync, 4, 0)
            # scalar: w0a, w1a, w2a, w3b, rw
            dma_half(nc.scalar, 0, 0)
            dma_half(nc.scalar, 1, 0)
            dma_half(nc.scalar, 2, 0)
            dma_half(nc.scalar, 3, 1)
            nc.scalar.dma_start(rw[:], rw_d[:])
            # gpsimd: w1b, w2b, w4b
            dma_half(nc.gpsimd, 1, 1)
            dma_half(nc.gpsimd, 2, 1)
            dma_half(nc.gpsimd, 4, 1)

            # paired accumulator: [128, 2, 4, 16, 8] so a 2-group product
            # pair is absorbed in ONE 1024-element add.  All adds stay on the
            # vector engine: gpsimd is strict-FIFO, so an add queued there
            # blocks the weight-DMA issues behind it and stalls the PE.
            acc_v = cpool.tile([128, 2, 4, 16, 8], f32)
            nc.vector.memset(acc_v[:], 0.0)

            # lead-in bytes already committed per queue (for the balancer)
            half_b = 128 * 2 * 8 * CSL * (1 if _W_DT_NAME == "float8e3" else 2)
            xt_b = 128 * 8 * 128 * 2
            rw_b = 128 * G * 4 * 8 * (2 if _RW_BF16 else 4)
            wq = _WeightedQueue(
                (xt_b + 3 * half_b, 4 * half_b + rw_b, 3 * half_b)
            )
            dma_engines = [nc.sync, nc.scalar, nc.gpsimd]

            def mult(dst, ps_ap, g, nrel=4, rel0=0):
                # dst = ps * rwsv[b, g, rel0:rel0+nrel, m] (broadcast over nl)
                in1 = rw[:, g, rel0 : rel0 + nrel]
                in1 = in1[:, :, None, :].to_broadcast([128, nrel, 16, 8])
                nc.vector.tensor_tensor(dst, ps_ap, in1, mybir.AluOpType.mult)

            # pair p = (g=2p, g=2p+1); the last pair is two solo adds so the
            # tail chain after the final matmul stays short.
            pair_t = None

            for g in range(G):
                slot = g % 2
                if slot == 0:
                    pair_t = tpool.tile([128, 2, 4, 16, 8], f32, tag="tmp")
                if g < LEADIN:
                    for h in range(2):
                        ps2 = pspool.tile(
                            [128, 2, 16, 8], f32, tag="ps", name=f"psl{g}_{h}"
                        )
                        for k in range(8):
                            nc.tensor.matmul(
                                ps2[:],
                                xt[:, k, :],
                                half[(g, h)][:, :, k, :],
                                start=(k == 0),
                                stop=(k == 7),
                            )
                        mult(pair_t[:, slot, 2 * h : 2 * h + 2], ps2[:], g, 2, 2 * h)
                else:
                    # one 512 KB transfer per group: half the DMA issues and
                    # completion semaphores of the two-half scheme (the deep
                    # prefetch makes the coarser gating irrelevant mid-kernel)
                    wt = wpool.tile([128, 4, 8, CSL], _W_DT, tag="wt")
                    q = dma_engines[wq.pick(2 * half_b)]
                    q.dma_start(wt[:], w_d[g])
                    ps = pspool.tile([128, 4, 16, 8], f32, tag="ps")
                    for k in range(8):
                        nc.tensor.matmul(
                            ps[:],
                            xt[:, k, :],
                            wt[:, :, k, :],
                            start=(k == 0),
                            stop=(k == 7),
                        )
                    mult(pair_t[:, slot], ps[:], g)
                p = g // 2
                if p == G // 2 - 1:
                    # last pair: solo adds into separate slots; slot 0's
                    # reduce + DMA overlap the final group's matmuls
                    nc.vector.tensor_tensor(
                        acc_v[:, slot], acc_v[:, slot], pair_t[:, slot],
                        mybir.AluOpType.add,
                    )
                    su_t = cpool.tile([128, 16], f32, name=f"su{slot}")
                    nc.vector.tensor_reduce(
                        su_t[:],
                        acc_v[:, slot].transpose([0, 2, 1, 3]),
                        mybir.AxisListType.XY,
                        mybir.AluOpType.add,
                    )
                    (nc.sync if slot == 0 else nc.scalar).dma_start(
                        su_d[slot], su_t[:]
                    )
                elif slot == 1:
                    nc.vector.tensor_tensor(
                        acc_v[:], acc_v[:], pair_t[:], mybir.AluOpType.add
                    )
                if g < G - 1:
                    fillers(_F_MID)

    if os.environ.get("BASS_STRIP_FRAMEWORK", "1") == "1":
        _strip_framework_overhead(nc)
    _split_multi_waits(nc)
    _NC_CACHE[key] = nc
    return nc


def _to_bf16(a):
    """Fast float32 -> bfloat16 with round-to-nearest-even (numpy bit ops;
    ml_dtypes astype is ~50x slower)."""
    u = a.view(np.uint32)
    r = ((u >> 16) & 1) + np.uint32(0x7FFF)
    return ((u + r) >> 16).astype(np.uint16).view(ml_dtypes.bfloat16)


def _quant_w(rel_W):
    """rel_W [N, I, O] f32 -> scaled float8e3 (or bf16 fallback)."""
    if _W_DT_NAME == "float8e3":
        z = np.clip(rel_W * np.float32(_W_SCALE), -15.5, 15.5)
        return z.astype(ml_dtypes.float8_e3m4)
    return _to_bf16(np.ascontiguousarray(rel_W, np.float32))


def _prep_core_w(w6, d):
    # w6: [G, 4, 8, 128, NC, CSL] quantized view -> (g, i_loc, r4, k, c)
    return np.ascontiguousarray(w6[:, :, :, :, d, :].transpose(0, 3, 1, 2, 4))


def kernel(x, edge_index, edge_type, rel_W, rel_b, route_weights):
    global LAST_RESULTS
    x = np.asarray(x, np.float32)
    rel_W = np.asarray(rel_W, np.float32)
    rel_b = np.asarray(rel_b, np.float32)
    rw = np.asarray(route_weights, np.float32).reshape(B, I, O)

    # host-side tiny reductions
    rwsum = rw.sum(axis=1, dtype=np.float32)                # [B, O]
    rwsv = np.ascontiguousarray(rwsum.reshape(B, 8, 128))   # [b, m, r]
    bias2 = np.einsum(
        "rnm,bmr->bn", rel_b.reshape(N, N, 8), rwsv, optimize=True
    )  # [B, N]

    # device input prep
    xt = _to_bf16(np.ascontiguousarray(x.reshape(B, 8, 128).transpose(2, 1, 0)))
    wq8 = _quant_w(rel_W)
    w6 = wq8.reshape(G, 4, 8, 128, NC, CSL)  # (g, r4, k, i_loc, d, c)
    with ThreadPoolExecutor(NC) as ex:
        w_cores = list(ex.map(lambda d: _prep_core_w(w6, d), range(NC)))

    # device rwsv in [b, g, r4, m] layout, with the weight scale folded in
    rwt = rwsum.reshape(B, 8, G, 4).transpose(0, 2, 3, 1)  # [b, g, r4, m]
    rwsv_dev = rwt * np.float32(1.0 / _W_SCALE if _W_DT_NAME == "float8e3" else 1.0)
    if _RW_BF16:
        rwsv_dev = _to_bf16(np.ascontiguousarray(rwsv_dev))
    else:
        rwsv_dev = np.ascontiguousarray(rwsv_dev)

    nc = _build_bass()
    in_maps = [{"xt": xt, "w": w_cores[d], "rwsv": rwsv_dev} for d in range(NC)]
    trace = bool(int(os.environ.get("KERNEL_TRACE", "0")))
    kwargs = {}
    if trace:
        _ensure_ntff_hook()
        kwargs["tmpdir"] = os.environ.get("KERNEL_TRACE_DIR") or tempfile.mkdtemp(
            prefix="capsule_trace_"
        )
    res = run_bass_kernel_spmd(nc, in_maps, list(range(NC)), trace=trace, **kwargs)
    LAST_RESULTS = res

    su = np.concatenate(
        [res.results[d]["su"].sum(axis=0, dtype=np.float32) for d in range(NC)],
        axis=1,
    )  # [B, N]
    su += bias2

    s = su * np.float32(1.0 / N)
    sn = np.sum(s * s, axis=-1, keepdims=True)
    vrow = (sn / (1.0 + sn) * s / np.sqrt(sn)).astype(np.float32)  # [B, N]
    out = np.empty((B, N, N), np.float32)
    out[:] = vrow[:, None, :]
    return out
